# revision 13
# baseline (speedup 1.0000x reference)
"""Trainium2 Bass kernel for a 2-layer GCN classifier (nn_GCNClassifier).

Reference computation (all f32):
    h1 = relu(adj1 @ x @ W1 + b1) + relu(adj2 @ x @ W1 + b1)   # [8192, 64]
    h2 = relu(adj1 @ h1 @ W2 + b2) + relu(adj2 @ h1 @ W2 + b2) # [8192, 16]

Sharding: 1D row partition of adj1/adj2 across 8 cores (1024 output rows per
core). Each core receives its adjacency row-shard PRE-TRANSPOSED on the host
(adj[rows, :].T, shape [8192, 1024], contiguous) so the contraction index
lands on the SBUF partition dim and every DMA line is 4KB contiguous.

On-chip layout is feature-major ("transposed space"): aggregates are computed
as aggT[f, m] = sum_k x[k, f] * adjT[k, m] with the tiny feature block as the
stationary matmul operand and the streaming adjacency as the moving operand.
Layer 2 uses associativity: adj @ (h1 @ W2), so only [8192, 16] crosses cores
via AllGather. The adjacency stream is carried in fp16 on the wire, halving
HBM traffic; the PE consumes it far below single-pass rate.

Key scheduling ideas on top of the plain stream:
 - The DMA stream is the bottleneck (~324 GB/s effective per core), so the
   last CACHE_A1+CACHE_A2 k-groups of each adjacency are DMA'd once in L1
   into persistent SBUF tiles and REUSED in L2 (no second read): L2 streams
   only 32 - CACHE MB. Cached groups are interleaved late in L2's matmul
   order so the PE chews them in the DMA shadow, and the final group is
   cached so the stream's tail collapses.
 - The inter-layer exchange is latency-compressed: one PSUM->SBUF copy for
   the L1 aggregate, z/h1 in 4 chunks, all 8 g-block matmuls into a single
   PSUM tile, one cast-copy, ONE 32KB g store, AllGather, 7 rotated gathers.
   The stream pool (9 bufs) absorbs the remaining collective latency.
 - Each core streams its OWN k-chunk first in layer 2 (host-rotated layouts
   + partition-id-offset gathers) so own-chunk PE work overlaps the
   AllGather latency.
 - Output is produced feature-major [16, 1024] and stored with a single
   DMA; the host transposes (free) - no PE transposes, no tiny stores.

Engine split: sync and scalar each issue one adjacency stream (one HWDGE
descriptor generator per stream); gpsimd issues all small DMAs (constants,
g bounce, gathers) and the collective.
"""

import numpy as np

import concourse.bacc as bacc
import concourse.bass as bass
import concourse.mybir as mybir
import concourse.tile as tile
from concourse.bass_utils import run_bass_kernel_spmd

N = 8192
IN_DIM, HID_DIM, OUT_DIM = 32, 64, 16
N_CORES = 8
ROWS = N // N_CORES          # 1024 output rows per core
KBLK = 128                   # contraction block (SBUF partition dim)
KMERGE = 4                   # k-blocks fetched per DMA (1MB tiles, 8KB lines)
NKB = N // KBLK              # 64 contraction blocks
NKG = NKB // KMERGE          # 16 merged DMA groups per adjacency
MCHUNK = 512                 # moving free-dim per matmul (PSUM bank limit)
NMC = ROWS // MCHUNK         # 2 m-chunks per core
F32 = mybir.dt.float32
FP16 = mybir.dt.float16
ADJ_DT = FP16
RELU = mybir.ActivationFunctionType.Relu

STREAM_BUFS = 9              # 1MB pool slots for the adjacency stream
CACHE_A1 = 6                 # trailing k-groups of adj1 kept in SBUF for L2
CACHE_A2 = 6                 # trailing k-groups of adj2 kept in SBUF for L2


def _merge(stream, cached):
    """Round-robin streamed/cached groups (cached consumed in the DMA
    shadow), always ending on a cached group when one exists."""
    head, tail = cached[:-1], cached[-1:]
    order = []
    si, ci = 0, 0
    while si < len(stream) or ci < len(head):
        if si < len(stream):
            order.append(stream[si]); si += 1
            if si < 2:
                continue
        if ci < len(head) and si >= 2:
            order.append(head[ci]); ci += 1
    return order + tail


def _l2_order(ncached):
    """Per-adjacency L2 k-group processing order: own chunk (kg 0,1) first
    (no AllGather dependency), then streamed groups with cached ones
    interleaved so the PE absorbs them in the DMA shadow."""
    stream = [kg for kg in range(2, NKG - ncached)]
    cached = [kg for kg in range(NKG - ncached, NKG)]
    return [0, 1] + _merge(stream, cached)


def _build_program():
    nc = bacc.Bacc(
        "TRN2", target_bir_lowering=False, debug=False, num_devices=N_CORES
    )
    a1t = nc.dram_tensor("a1t", [NKG, KBLK, KMERGE, ROWS], ADJ_DT, kind="ExternalInput")
    a2t = nc.dram_tensor("a2t", [NKG, KBLK, KMERGE, ROWS], ADJ_DT, kind="ExternalInput")
    featb = nc.dram_tensor("featb", [KBLK, NKB, IN_DIM], ADJ_DT, kind="ExternalInput")
    w1 = nc.dram_tensor("w1", [IN_DIM, HID_DIM], F32, kind="ExternalInput")
    b1 = nc.dram_tensor("b1", [HID_DIM, 1], F32, kind="ExternalInput")
    w2 = nc.dram_tensor("w2", [HID_DIM, OUT_DIM], F32, kind="ExternalInput")
    b2 = nc.dram_tensor("b2", [OUT_DIM, 1], F32, kind="ExternalInput")
    out = nc.dram_tensor("out", [OUT_DIM, ROWS], F32, kind="ExternalOutput")

    with tile.TileContext(nc) as tc:
        _kernel_body(nc, tc, a1t, a2t, featb, w1, b1, w2, b2, out)
    nc.compile()
    return nc


def _aggregate(nc, adjp, psp, adj_drams, lhs_blocks, fdim, tag, cache, fill):
    """aggT[a][f, m] = sum_k lhs[k, f] * adjT[a][k, m] for both adjacencies.

    cache: dict (ai, kg) -> persistent SBUF tile for reused groups.
    fill=True (L1): every group is DMA'd (cached groups into their
    persistent tile); natural ascending order.
    fill=False (L2): cached groups skip the DMA; interleaved order.
    Returns the shared PSUM acc tile (4 col-packed accumulations).
    """
    acc = psp.tile([128, MCHUNK], F32, tag="accm", name=f"{tag}m")
    if fill:
        orders = [list(range(NKG)), list(range(NKG))]
    else:
        orders = [_l2_order(CACHE_A1), _l2_order(CACHE_A2)]
    # single HWDGE queue for the whole stream (one sequential HBM access
    # pattern); flags per (ai): start on first kb, stop on last kb
    firsts = [o[0] * KMERGE for o in orders]
    lasts = [o[-1] * KMERGE + KMERGE - 1 for o in orders]
    for step in range(NKG):
        for ai, adj in enumerate(adj_drams):
            kg = orders[ai][step]
            cached = (ai, kg) in cache
            if cached:
                at = cache[(ai, kg)]
                if fill:
                    nc.sync.dma_start(at[:], adj[kg])
            else:
                at = adjp.tile(
                    [KBLK, KMERGE, ROWS], ADJ_DT, tag="adj",
                    name=f"{tag}_adj{ai}_{kg}",
                )
                if fill and step == 0 and ai == 0:
                    # split the very first transfer so data starts flowing
                    # before the full 1MB descriptor set is generated
                    half = KMERGE // 2
                    nc.sync.dma_start(at[:, :half, :], adj[kg, :, :half, :])
                    nc.sync.dma_start(at[:, half:, :], adj[kg, :, half:, :])
                else:
                    nc.sync.dma_start(at[:], adj[kg])
            for t in range(KMERGE):
                kb = kg * KMERGE + t
                lhs = lhs_blocks(kb)
                for mc in range(NMC):
                    off = (ai * NMC + mc) * 32
                    nc.tensor.matmul(
                        acc[off:off + fdim, :],
                        lhs,
                        at[:, t, mc * MCHUNK:(mc + 1) * MCHUNK],
                        start=(kb == firsts[ai]),
                        stop=(kb == lasts[ai]),
                        tile_position=(0, off),
                    )
    return acc


def _kernel_body(nc, tc, a1t, a2t, featb, w1, b1, w2, b2, out):
    with (
        tc.tile_pool(name="const", bufs=1) as constp,
        tc.tile_pool(name="cache", bufs=1) as cachep,
        tc.tile_pool(name="adj", bufs=STREAM_BUFS) as adjp,
        tc.tile_pool(name="work", bufs=1) as workp,
        tc.tile_pool(name="psum", bufs=1, space="PSUM") as psp,
        tc.tile_pool(name="dram", bufs=1, space="DRAM") as dramp,
    ):
        # --- tiny warm-up AllGather first: wakes the ncfw/CC path during the
        # L1 stream so the real inter-layer exchange doesn't pay first-use
        # setup, and acts as an early cross-core alignment barrier
        warm_sb = constp.tile([1, N_CORES], F32)
        nc.gpsimd.memset(warm_sb[:], 0.0)
        warm_in = dramp.tile([1, N_CORES], F32)
        warm_out = dramp.tile([N_CORES, N_CORES], F32, addr_space="Shared")
        nc.gpsimd.dma_start(warm_in[:], warm_sb[:])
        nc.gpsimd.collective_compute(
            "AllGather",
            mybir.AluOpType.bypass,
            replica_groups=[list(range(N_CORES))],
            ins=[warm_in.opt()],
            outs=[warm_out.opt()],
        )

        # --- constants on gpsimd (SWDGE) so the sync/scalar HWDGE queues
        # start the adjacency stream immediately
        xb = constp.tile([KBLK, NKB, IN_DIM], ADJ_DT)   # features, k-blocked
        nc.gpsimd.dma_start(xb[:], featb[:])
        # W1 replicated at partition offsets 0/32/64/96 so each z-matmul can
        # read its aggregate chunk straight out of the packed accumulator
        # copy (PE row group = chunk's partition offset via tile_position)
        w1_sb = constp.tile([128, HID_DIM], F32)
        for g in range(4):
            nc.gpsimd.dma_start(w1_sb[g * 32:g * 32 + IN_DIM, :], w1[:])
        b1_sb = constp.tile([HID_DIM, 1], F32)
        nc.gpsimd.dma_start(b1_sb[:], b1[:])
        w2_sb = constp.tile([HID_DIM, OUT_DIM], F32)
        nc.gpsimd.dma_start(w2_sb[:], w2[:])
        b2_sb = constp.tile([OUT_DIM, 1], F32)
        nc.gpsimd.dma_start(b2_sb[:], b2[:])

        # persistent cache tiles: trailing k-groups of each adjacency stay
        # in SBUF after L1 so L2 never re-reads them from HBM
        cache = {}
        for ai, ckg in ((0, CACHE_A1), (1, CACHE_A2)):
            for kg in range(NKG - ckg, NKG):
                cache[(ai, kg)] = cachep.tile(
                    [KBLK, KMERGE, ROWS], ADJ_DT, name=f"csh{ai}_{kg}"
                )

        # --- layer 1: aggT = (adjT)^T-contract with x blocks ---
        acc1 = _aggregate(
            nc, adjp, psp, (a1t, a2t), lambda kb: xb[:, kb, :], IN_DIM,
            "l1", cache, fill=True,
        )
        # L1 epilogue, pipelined per m-chunk so the first half-AllGather
        # launches as early as possible: for each mc, z = W1^T @ agg chunks
        # (PE row groups via tile_position), relu, add branches, 4 g-block
        # matmuls into one PSUM tile, one cast-copy, ONE 16KB store, and the
        # half-collective. Even k-groups of L2 only need the first half.
        asb = workp.tile([128, MCHUNK], F32)
        h1_parts = [
            workp.tile([HID_DIM, ROWS], F32, name=f"h1p{ai}") for ai in range(2)
        ]
        h1T = workp.tile([HID_DIM, ROWS], F32)
        nloc = ROWS // KBLK                              # 8 local k-blocks
        nhalf = nloc // NMC                              # g blocks per half
        g_ps = psp.tile([KBLK, nloc, OUT_DIM], F32, tag="gg")
        g_sb = workp.tile([KBLK, nloc, OUT_DIM], ADJ_DT)
        for mc in range(NMC):
            csl = slice(mc * MCHUNK, (mc + 1) * MCHUNK)
            for ai in range(2):
                off = (ai * NMC + mc) * 32
                nc.vector.tensor_copy(
                    asb[off:off + IN_DIM, :], acc1[off:off + IN_DIM, :]
                )
                z_ps = psp.tile([HID_DIM, MCHUNK], F32, tag="zz", bufs=2)
                nc.tensor.matmul(
                    z_ps[:],
                    w1_sb[off:off + IN_DIM, :],
                    asb[off:off + IN_DIM, :],
                    start=True,
                    stop=True,
                    tile_position=(off, 0),
                )
                nc.scalar.activation(
                    h1_parts[ai][:, csl], z_ps[:], RELU, bias=b1_sb[:]
                )
            nc.vector.tensor_add(
                h1T[:, csl], h1_parts[0][:, csl], h1_parts[1][:, csl]
            )
            hsl = slice(mc * nhalf, (mc + 1) * nhalf)
            for i in range(mc * nhalf, (mc + 1) * nhalf):
                nc.tensor.matmul(
                    g_ps[:, i, :],
                    h1T[:, i * KBLK:(i + 1) * KBLK],
                    w2_sb[:],
                    start=True,
                    stop=True,
                )
            nc.vector.tensor_copy(g_sb[:, hsl, :], g_ps[:, hsl, :])

        # ONE 32KB store on the (otherwise idle) scalar HWDGE queue, one
        # AllGather, then rotated gathers - minimal exchange latency
        g_loc = dramp.tile([KBLK, nloc * OUT_DIM], ADJ_DT)
        nc.scalar.dma_start(g_loc[:], g_sb[:].rearrange("p j o -> p (j o)"))
        g_cat = dramp.tile([N_CORES * KBLK, nloc * OUT_DIM], ADJ_DT,
                           addr_space="Shared")
        nc.gpsimd.collective_compute(
            "AllGather",
            mybir.AluOpType.bypass,
            replica_groups=[list(range(N_CORES))],
            ins=[g_loc.opt()],
            outs=[g_cat.opt()],
        )
        # Remote g chunks, gathered in per-core rotated order: stream block s
        # covers global chunk (pid + s//nloc) mod 8, so every core streams
        # its OWN chunk first (lhs straight from g_sb, no AllGather dep).
        gb2 = constp.tile([KBLK, NKB - nloc, OUT_DIM], ADJ_DT)
        pid = nc.scalar.partition_id()
        for j in range(1, N_CORES):
            q = (pid + j) & (N_CORES - 1)
            nc.scalar.dma_start(
                gb2[:, (j - 1) * nloc:j * nloc, :],
                g_cat[bass.ds(q * KBLK, KBLK), :]
                .rearrange("p (j2 o) -> p j2 o", j2=nloc),
            )

        def l2_lhs(kb):
            return g_sb[:, kb, :] if kb < nloc else gb2[:, kb - nloc, :]

        # --- layer 2: agg2T = contract adjT with g blocks ---
        acc2 = _aggregate(
            nc, adjp, psp, (a1t, a2t), l2_lhs, OUT_DIM,
            "l2", cache, fill=False,
        )

        # h2T = relu(agg2T + b2) summed over branches, stored feature-major
        # with a single DMA per m-chunk (host transposes for free)
        h2_parts = [
            workp.tile([OUT_DIM, ROWS], F32, name=f"h2p{ai}") for ai in range(2)
        ]
        h2T = workp.tile([OUT_DIM, ROWS], F32)
        oeng = (nc.sync, nc.scalar)
        for mc in range(NMC):
            sl = slice(mc * MCHUNK, (mc + 1) * MCHUNK)
            for ai in range(2):
                off = (ai * NMC + mc) * 32
                nc.scalar.activation(
                    h2_parts[ai][:, sl],
                    acc2[off:off + OUT_DIM, :],
                    RELU,
                    bias=b2_sb[:],
                )
            nc.vector.tensor_add(
                h2T[:, sl], h2_parts[0][:, sl], h2_parts[1][:, sl]
            )
            oeng[mc % 2].dma_start(out[:, sl], h2T[:, sl])


_NC_CACHE = None


def _get_nc():
    global _NC_CACHE
    if _NC_CACHE is None:
        _NC_CACHE = _build_program()
    return _NC_CACHE


def _shard_inputs(inputs):
    wire_np = np.float16
    adj1 = np.asarray(inputs["adj1"], dtype=np.float32)
    adj2 = np.asarray(inputs["adj2"], dtype=np.float32)
    feat = np.asarray(inputs["features"], dtype=np.float32)
    featb0 = np.ascontiguousarray(
        feat.reshape(NKB, KBLK, IN_DIM).swapaxes(0, 1)
    ).astype(wire_np)
    w1 = np.ascontiguousarray(inputs["W1"], dtype=np.float32)
    b1 = np.ascontiguousarray(inputs["b1"], dtype=np.float32).reshape(HID_DIM, 1)
    w2 = np.ascontiguousarray(inputs["W2"], dtype=np.float32)
    b2 = np.ascontiguousarray(inputs["b2"], dtype=np.float32).reshape(OUT_DIM, 1)
    in_maps = []
    for c in range(N_CORES):
        rows = slice(c * ROWS, (c + 1) * ROWS)
        # per-core rotation: stream this core's own k-chunk (blocks 8c..8c+7)
        # first; matches the kernel's (pid + s//8) mod 8 gather order
        featb = np.ascontiguousarray(np.roll(featb0, -c * ROWS // KBLK, axis=1))

        # blocked-transposed: [kg, p, t, m] = adj[c*ROWS + m, kg*KM*128 + t*128 + p]
        def blockT(a):
            blocked = (
                a[rows, :]
                .reshape(ROWS, NKG, KMERGE, KBLK)
                .transpose(1, 3, 2, 0)
                .astype(wire_np)
            )
            ngrp_per_core = ROWS // (KMERGE * KBLK)      # groups per chunk
            return np.ascontiguousarray(
                np.roll(blocked, -c * ngrp_per_core, axis=0)
            )
        in_maps.append({
            "a1t": blockT(adj1),
            "a2t": blockT(adj2),
            "featb": featb,
            "w1": w1,
            "b1": b1,
            "w2": w2,
            "b2": b2,
        })
    return in_maps


def _ensure_ntff_shim():
    # bass_utils' axon trace path imports antenv.axon_hooks, which this agent
    # image lacks; stub it so a stray BASS_TRACE=1 env can't crash the run.
    import sys as _sys
    try:
        import antenv.axon_hooks  # noqa: F401
    except ImportError:
        import types as _types
        mod = _types.ModuleType("antenv.axon_hooks")
        _state = {"hook": None}
        mod.set_axon_ntff_profile_hook = lambda h: _state.__setitem__("hook", h)
        mod.get_axon_ntff_profile_hook = lambda: _state["hook"]
        _sys.modules["antenv.axon_hooks"] = mod


def _run(inputs, trace=False, trace_cores=None, stitch_traces=False):
    _ensure_ntff_shim()
    nc = _get_nc()
    in_maps = _shard_inputs(inputs)
    res = run_bass_kernel_spmd(
        nc,
        in_maps,
        core_ids=list(range(N_CORES)),
        trace=trace,
        trace_cores=trace_cores,
        stitch_traces=stitch_traces,
    )
    full = np.concatenate(
        [res.results[c]["out"].T for c in range(N_CORES)], axis=0
    ).astype(np.float32)
    return full, res


def kernel(**inputs):
    full, _ = _run(inputs, trace=False)
    return full


# revision 16
# speedup vs baseline: 1.0335x; 1.0335x over previous
"""Trainium2 Bass kernel for a 2-layer GCN classifier (nn_GCNClassifier).

Reference computation (all f32):
    h1 = relu(adj1 @ x @ W1 + b1) + relu(adj2 @ x @ W1 + b1)   # [8192, 64]
    h2 = relu(adj1 @ h1 @ W2 + b2) + relu(adj2 @ h1 @ W2 + b2) # [8192, 16]

Sharding: 1D row partition of adj1/adj2 across 8 cores (1024 output rows per
core). Each core receives its adjacency row-shard PRE-TRANSPOSED on the host
(adj[rows, :].T, shape [8192, 1024], contiguous) so the contraction index
lands on the SBUF partition dim and every DMA line is 4KB contiguous.

On-chip layout is feature-major ("transposed space"): aggregates are computed
as aggT[f, m] = sum_k x[k, f] * adjT[k, m] with the tiny feature block as the
stationary matmul operand and the streaming adjacency as the moving operand.
Layer 2 uses associativity: adj @ (h1 @ W2), so only [8192, 16] crosses cores
via AllGather. The adjacency stream is carried in fp16 on the wire, halving
HBM traffic; the PE consumes it far below single-pass rate.

Key scheduling ideas on top of the plain stream:
 - The DMA stream is the bottleneck (~324 GB/s effective per core), so the
   last CACHE_A1+CACHE_A2 k-groups of each adjacency are DMA'd once in L1
   into persistent SBUF tiles and REUSED in L2 (no second read): L2 streams
   only 32 - CACHE MB. Cached groups are interleaved late in L2's matmul
   order so the PE chews them in the DMA shadow, and the final group is
   cached so the stream's tail collapses.
 - The inter-layer exchange is latency-compressed: one PSUM->SBUF copy for
   the L1 aggregate, z/h1 in 4 chunks, all 8 g-block matmuls into a single
   PSUM tile, one cast-copy, ONE 32KB g store, AllGather, 7 rotated gathers.
   The stream pool (9 bufs) absorbs the remaining collective latency.
 - Each core streams its OWN k-chunk first in layer 2 (host-rotated layouts
   + partition-id-offset gathers) so own-chunk PE work overlaps the
   AllGather latency.
 - Output is produced feature-major [16, 1024] and stored with a single
   DMA; the host transposes (free) - no PE transposes, no tiny stores.

Engine split: sync and scalar each issue one adjacency stream (one HWDGE
descriptor generator per stream); gpsimd issues all small DMAs (constants,
g bounce, gathers) and the collective.
"""

import numpy as np

import concourse.bacc as bacc
import concourse.bass as bass
import concourse.mybir as mybir
import concourse.tile as tile
from concourse.bass_utils import run_bass_kernel_spmd

N = 8192
IN_DIM, HID_DIM, OUT_DIM = 32, 64, 16
N_CORES = 8
ROWS = N // N_CORES          # 1024 output rows per core
KBLK = 128                   # contraction block (SBUF partition dim)
KMERGE = 4                   # k-blocks fetched per DMA (1MB tiles, 8KB lines)
NKB = N // KBLK              # 64 contraction blocks
NKG = NKB // KMERGE          # 16 merged DMA groups per adjacency
MCHUNK = 512                 # moving free-dim per matmul (PSUM bank limit)
NMC = ROWS // MCHUNK         # 2 m-chunks per core
F32 = mybir.dt.float32
FP16 = mybir.dt.float16
ADJ_DT = FP16
RELU = mybir.ActivationFunctionType.Relu

STREAM_BUFS = 9              # 1MB pool slots for the adjacency stream
CACHE_A1 = 6                 # trailing k-groups of adj1 kept in SBUF for L2
CACHE_A2 = 6                 # trailing k-groups of adj2 kept in SBUF for L2


def _merge(stream, cached):
    """Round-robin streamed/cached groups (cached consumed in the DMA
    shadow), always ending on a cached group when one exists."""
    head, tail = cached[:-1], cached[-1:]
    order = []
    si, ci = 0, 0
    while si < len(stream) or ci < len(head):
        if si < len(stream):
            order.append(stream[si]); si += 1
            if si < 2:
                continue
        if ci < len(head) and si >= 2:
            order.append(head[ci]); ci += 1
    return order + tail


def _l2_order(ncached):
    """Per-adjacency L2 k-group processing order: own chunk (kg 0,1) first
    (no AllGather dependency), then streamed groups with cached ones
    interleaved so the PE absorbs them in the DMA shadow."""
    stream = [kg for kg in range(2, NKG - ncached)]
    cached = [kg for kg in range(NKG - ncached, NKG)]
    return [0, 1] + _merge(stream, cached)


def _build_program():
    nc = bacc.Bacc(
        "TRN2", target_bir_lowering=False, debug=False, num_devices=N_CORES
    )
    a1t = nc.dram_tensor("a1t", [NKG, KBLK, KMERGE, ROWS], ADJ_DT, kind="ExternalInput")
    a2t = nc.dram_tensor("a2t", [NKG, KBLK, KMERGE, ROWS], ADJ_DT, kind="ExternalInput")
    featb = nc.dram_tensor("featb", [KBLK, NKB, IN_DIM], ADJ_DT, kind="ExternalInput")
    w1 = nc.dram_tensor("w1", [IN_DIM, HID_DIM], F32, kind="ExternalInput")
    b1 = nc.dram_tensor("b1", [HID_DIM, 1], F32, kind="ExternalInput")
    w2 = nc.dram_tensor("w2", [HID_DIM, OUT_DIM], F32, kind="ExternalInput")
    b2 = nc.dram_tensor("b2", [OUT_DIM, 1], F32, kind="ExternalInput")
    out = nc.dram_tensor("out", [OUT_DIM, ROWS], F32, kind="ExternalOutput")

    with tile.TileContext(nc) as tc:
        _kernel_body(nc, tc, a1t, a2t, featb, w1, b1, w2, b2, out)
    nc.compile()
    return nc


def _aggregate(nc, adjp, psp, adj_drams, lhs_blocks, fdim, tag, cache, fill):
    """aggT[a][f, m] = sum_k lhs[k, f] * adjT[a][k, m] for both adjacencies.

    cache: dict (ai, kg) -> persistent SBUF tile for reused groups.
    fill=True (L1): every group is DMA'd (cached groups into their
    persistent tile); natural ascending order.
    fill=False (L2): cached groups skip the DMA; interleaved order.
    Returns the shared PSUM acc tile (4 col-packed accumulations).
    """
    acc = psp.tile([128, MCHUNK], F32, tag="accm", name=f"{tag}m")
    dma_engines = (nc.sync, nc.scalar)   # one HWDGE generator per adjacency
    if fill:
        orders = [list(range(NKG)), list(range(NKG))]
    else:
        orders = [_l2_order(CACHE_A1), _l2_order(CACHE_A2)]
    # flags per (ai): start on first kb, stop on last kb
    firsts = [o[0] * KMERGE for o in orders]
    lasts = [o[-1] * KMERGE + KMERGE - 1 for o in orders]
    for step in range(NKG):
        for ai, adj in enumerate(adj_drams):
            eng = dma_engines[ai]
            kg = orders[ai][step]
            cached = (ai, kg) in cache
            if cached:
                at = cache[(ai, kg)]
                if fill:
                    eng.dma_start(at[:], adj[kg])
            else:
                at = adjp.tile(
                    [KBLK, KMERGE, ROWS], ADJ_DT, tag="adj",
                    name=f"{tag}_adj{ai}_{kg}",
                )
                if fill and step == 0:
                    # split the very first transfer so data starts flowing
                    # before the full 1MB descriptor set is generated
                    half = KMERGE // 2
                    eng.dma_start(at[:, :half, :], adj[kg, :, :half, :])
                    eng.dma_start(at[:, half:, :], adj[kg, :, half:, :])
                else:
                    eng.dma_start(at[:], adj[kg])
            for t in range(KMERGE):
                kb = kg * KMERGE + t
                lhs = lhs_blocks(kb)
                for mc in range(NMC):
                    off = (ai * NMC + mc) * 32
                    nc.tensor.matmul(
                        acc[off:off + fdim, :],
                        lhs,
                        at[:, t, mc * MCHUNK:(mc + 1) * MCHUNK],
                        start=(kb == firsts[ai]),
                        stop=(kb == lasts[ai]),
                        tile_position=(0, off),
                    )
    return acc


def _kernel_body(nc, tc, a1t, a2t, featb, w1, b1, w2, b2, out):
    with (
        tc.tile_pool(name="const", bufs=1) as constp,
        tc.tile_pool(name="cache", bufs=1) as cachep,
        tc.tile_pool(name="adj", bufs=STREAM_BUFS) as adjp,
        tc.tile_pool(name="work", bufs=1) as workp,
        tc.tile_pool(name="psum", bufs=1, space="PSUM") as psp,
        tc.tile_pool(name="dram", bufs=1, space="DRAM") as dramp,
    ):
        # --- tiny warm-up AllGather first: wakes the ncfw/CC path during the
        # L1 stream so the real inter-layer exchange doesn't pay first-use
        # setup, and acts as an early cross-core alignment barrier
        warm_sb = constp.tile([1, N_CORES], F32)
        nc.gpsimd.memset(warm_sb[:], 0.0)
        warm_in = dramp.tile([1, N_CORES], F32)
        warm_out = dramp.tile([N_CORES, N_CORES], F32, addr_space="Shared")
        nc.gpsimd.dma_start(warm_in[:], warm_sb[:])
        nc.gpsimd.collective_compute(
            "AllGather",
            mybir.AluOpType.bypass,
            replica_groups=[list(range(N_CORES))],
            ins=[warm_in.opt()],
            outs=[warm_out.opt()],
        )

        # --- constants on gpsimd (SWDGE) so the sync/scalar HWDGE queues
        # start the adjacency stream immediately
        xb = constp.tile([KBLK, NKB, IN_DIM], ADJ_DT)   # features, k-blocked
        nc.gpsimd.dma_start(xb[:], featb[:])
        # W1 replicated at partition offsets 0/32/64/96 so each z-matmul can
        # read its aggregate chunk straight out of the packed accumulator
        # copy (PE row group = chunk's partition offset via tile_position)
        w1_sb = constp.tile([128, HID_DIM], F32)
        for g in range(4):
            nc.gpsimd.dma_start(w1_sb[g * 32:g * 32 + IN_DIM, :], w1[:])
        b1_sb = constp.tile([HID_DIM, 1], F32)
        nc.gpsimd.dma_start(b1_sb[:], b1[:])
        w2_sb = constp.tile([HID_DIM, OUT_DIM], F32)
        nc.gpsimd.dma_start(w2_sb[:], w2[:])
        b2_sb = constp.tile([OUT_DIM, 1], F32)
        nc.gpsimd.dma_start(b2_sb[:], b2[:])

        # persistent cache tiles: trailing k-groups of each adjacency stay
        # in SBUF after L1 so L2 never re-reads them from HBM
        cache = {}
        for ai, ckg in ((0, CACHE_A1), (1, CACHE_A2)):
            for kg in range(NKG - ckg, NKG):
                cache[(ai, kg)] = cachep.tile(
                    [KBLK, KMERGE, ROWS], ADJ_DT, name=f"csh{ai}_{kg}"
                )

        # --- layer 1: aggT = (adjT)^T-contract with x blocks ---
        acc1 = _aggregate(
            nc, adjp, psp, (a1t, a2t), lambda kb: xb[:, kb, :], IN_DIM,
            "l1", cache, fill=True,
        )
        # L1 epilogue, pipelined per m-chunk so the first half-AllGather
        # launches as early as possible: for each mc, z = W1^T @ agg chunks
        # (PE row groups via tile_position), relu, add branches, 4 g-block
        # matmuls into one PSUM tile, one cast-copy, ONE 16KB store, and the
        # half-collective. Even k-groups of L2 only need the first half.
        asb = workp.tile([128, MCHUNK], F32)
        h1_parts = [
            workp.tile([HID_DIM, ROWS], F32, name=f"h1p{ai}") for ai in range(2)
        ]
        h1T = workp.tile([HID_DIM, ROWS], F32)
        nloc = ROWS // KBLK                              # 8 local k-blocks
        nhalf = nloc // NMC                              # g blocks per half
        g_ps = psp.tile([KBLK, nloc, OUT_DIM], F32, tag="gg")
        g_sb = workp.tile([KBLK, nloc, OUT_DIM], ADJ_DT)
        for mc in range(NMC):
            csl = slice(mc * MCHUNK, (mc + 1) * MCHUNK)
            for ai in range(2):
                off = (ai * NMC + mc) * 32
                nc.vector.tensor_copy(
                    asb[off:off + IN_DIM, :], acc1[off:off + IN_DIM, :]
                )
                z_ps = psp.tile([HID_DIM, MCHUNK], F32, tag="zz", bufs=2)
                nc.tensor.matmul(
                    z_ps[:],
                    w1_sb[off:off + IN_DIM, :],
                    asb[off:off + IN_DIM, :],
                    start=True,
                    stop=True,
                    tile_position=(off, 0),
                )
                nc.scalar.activation(
                    h1_parts[ai][:, csl], z_ps[:], RELU, bias=b1_sb[:]
                )
            nc.vector.tensor_add(
                h1T[:, csl], h1_parts[0][:, csl], h1_parts[1][:, csl]
            )
            hsl = slice(mc * nhalf, (mc + 1) * nhalf)
            for i in range(mc * nhalf, (mc + 1) * nhalf):
                nc.tensor.matmul(
                    g_ps[:, i, :],
                    h1T[:, i * KBLK:(i + 1) * KBLK],
                    w2_sb[:],
                    start=True,
                    stop=True,
                )
            nc.vector.tensor_copy(g_sb[:, hsl, :], g_ps[:, hsl, :])

        # ONE 32KB store (gpsimd SWDGE - the HWDGE queues are mid-stream),
        # one AllGather, then rotated gathers - minimal exchange latency
        g_loc = dramp.tile([KBLK, nloc * OUT_DIM], ADJ_DT)
        nc.gpsimd.dma_start(g_loc[:], g_sb[:].rearrange("p j o -> p (j o)"))
        g_cat = dramp.tile([N_CORES * KBLK, nloc * OUT_DIM], ADJ_DT,
                           addr_space="Shared")
        nc.gpsimd.collective_compute(
            "AllGather",
            mybir.AluOpType.bypass,
            replica_groups=[list(range(N_CORES))],
            ins=[g_loc.opt()],
            outs=[g_cat.opt()],
        )
        # Remote g chunks, gathered in per-core rotated order: stream block s
        # covers global chunk (pid + s//nloc) mod 8, so every core streams
        # its OWN chunk first (lhs straight from g_sb, no AllGather dep).
        gb2 = constp.tile([KBLK, NKB - nloc, OUT_DIM], ADJ_DT)
        pid = nc.gpsimd.partition_id()
        for j in range(1, N_CORES):
            q = (pid + j) & (N_CORES - 1)
            nc.gpsimd.dma_start(
                gb2[:, (j - 1) * nloc:j * nloc, :],
                g_cat[bass.ds(q * KBLK, KBLK), :]
                .rearrange("p (j2 o) -> p j2 o", j2=nloc),
            )

        def l2_lhs(kb):
            return g_sb[:, kb, :] if kb < nloc else gb2[:, kb - nloc, :]

        # --- layer 2: agg2T = contract adjT with g blocks ---
        acc2 = _aggregate(
            nc, adjp, psp, (a1t, a2t), l2_lhs, OUT_DIM,
            "l2", cache, fill=False,
        )

        # h2T = relu(agg2T + b2) summed over branches, stored feature-major
        # with a single DMA per m-chunk (host transposes for free)
        h2_parts = [
            workp.tile([OUT_DIM, ROWS], F32, name=f"h2p{ai}") for ai in range(2)
        ]
        h2T = workp.tile([OUT_DIM, ROWS], F32)
        oeng = (nc.sync, nc.scalar)
        for mc in range(NMC):
            sl = slice(mc * MCHUNK, (mc + 1) * MCHUNK)
            for ai in range(2):
                off = (ai * NMC + mc) * 32
                nc.scalar.activation(
                    h2_parts[ai][:, sl],
                    acc2[off:off + OUT_DIM, :],
                    RELU,
                    bias=b2_sb[:],
                )
            nc.vector.tensor_add(
                h2T[:, sl], h2_parts[0][:, sl], h2_parts[1][:, sl]
            )
            oeng[mc % 2].dma_start(out[:, sl], h2T[:, sl])


_NC_CACHE = None


def _get_nc():
    global _NC_CACHE
    if _NC_CACHE is None:
        _NC_CACHE = _build_program()
    return _NC_CACHE


def _shard_inputs(inputs):
    wire_np = np.float16
    adj1 = np.asarray(inputs["adj1"], dtype=np.float32)
    adj2 = np.asarray(inputs["adj2"], dtype=np.float32)
    feat = np.asarray(inputs["features"], dtype=np.float32)
    featb0 = np.ascontiguousarray(
        feat.reshape(NKB, KBLK, IN_DIM).swapaxes(0, 1)
    ).astype(wire_np)
    w1 = np.ascontiguousarray(inputs["W1"], dtype=np.float32)
    b1 = np.ascontiguousarray(inputs["b1"], dtype=np.float32).reshape(HID_DIM, 1)
    w2 = np.ascontiguousarray(inputs["W2"], dtype=np.float32)
    b2 = np.ascontiguousarray(inputs["b2"], dtype=np.float32).reshape(OUT_DIM, 1)
    in_maps = []
    for c in range(N_CORES):
        rows = slice(c * ROWS, (c + 1) * ROWS)
        # per-core rotation: stream this core's own k-chunk (blocks 8c..8c+7)
        # first; matches the kernel's (pid + s//8) mod 8 gather order
        featb = np.ascontiguousarray(np.roll(featb0, -c * ROWS // KBLK, axis=1))

        # blocked-transposed: [kg, p, t, m] = adj[c*ROWS + m, kg*KM*128 + t*128 + p]
        def blockT(a):
            blocked = (
                a[rows, :]
                .reshape(ROWS, NKG, KMERGE, KBLK)
                .transpose(1, 3, 2, 0)
                .astype(wire_np)
            )
            ngrp_per_core = ROWS // (KMERGE * KBLK)      # groups per chunk
            return np.ascontiguousarray(
                np.roll(blocked, -c * ngrp_per_core, axis=0)
            )
        in_maps.append({
            "a1t": blockT(adj1),
            "a2t": blockT(adj2),
            "featb": featb,
            "w1": w1,
            "b1": b1,
            "w2": w2,
            "b2": b2,
        })
    return in_maps


def _ensure_ntff_shim():
    # bass_utils' axon trace path imports antenv.axon_hooks, which this agent
    # image lacks; stub it so a stray BASS_TRACE=1 env can't crash the run.
    import sys as _sys
    try:
        import antenv.axon_hooks  # noqa: F401
    except ImportError:
        import types as _types
        mod = _types.ModuleType("antenv.axon_hooks")
        _state = {"hook": None}
        mod.set_axon_ntff_profile_hook = lambda h: _state.__setitem__("hook", h)
        mod.get_axon_ntff_profile_hook = lambda: _state["hook"]
        _sys.modules["antenv.axon_hooks"] = mod


def _run(inputs, trace=False, trace_cores=None, stitch_traces=False):
    _ensure_ntff_shim()
    nc = _get_nc()
    in_maps = _shard_inputs(inputs)
    res = run_bass_kernel_spmd(
        nc,
        in_maps,
        core_ids=list(range(N_CORES)),
        trace=trace,
        trace_cores=trace_cores,
        stitch_traces=stitch_traces,
    )
    full = np.concatenate(
        [res.results[c]["out"].T for c in range(N_CORES)], axis=0
    ).astype(np.float32)
    return full, res


def kernel(**inputs):
    full, _ = _run(inputs, trace=False)
    return full
